# revision 1
# baseline (speedup 1.0000x reference)
"""Trainium2 Bass kernel for nn_DeltaSynapse.

I[b,o] = einsum('beo,dbe,deo,dbe->bo', Weff, Xd, delaymap, Wshort+1)
with Weff[b,e,o] = signs[e,o] * (W[e,o]*(1-frac[e,o]) + Wlong[b,e,o]*frac[e,o])
     signs[e,o] = sign(signs_pre[e]) * (W[e,o] > 0)

Rewrite: G[d,b,e] = Xd*(Wshort+1), A = signs*W*(1-frac), SF = signs*frac.
  I[b,o] = sum_{d,e} G[d,b,e]*A[e,o]*dm[d,e,o]                 (term1: matmul)
         + sum_e Wlong[b,e,o] * H[b,e,o],                      (term2)
  H[b,e,o] = sum_d G[d,b,e]*Q[d,e,o],  Q = SF*dm.

Sharding: o (post) dim across 8 cores (No=256 each) -> every HBM byte read
once. Per core, per e-group g of J=16 e's (NG = N/J groups):
  - P = arep*dm_t, Q = srep*dm_t  (DVE / GPSIMD bf16 tensor_mul, 2x mode)
  - term1 MM: I_psum[16,No] += gq1[:,g,:].T @ P     (K=(d,j)=128, M=B)
  - H' MM per b-half: Hp[128,No] = gblk_half.T @ Q  (block-diag stationary:
      lhsT[(d,j),(bb,j')] = G[d,b,g*16+j]*delta_{j,j'} -> M=(bb,j')=128 full)
  - Z = wl_t * Hs  (Hs = ACT evac of Hp PSUM->SBUF bf16)
  - Zred MM: I_psum += eh[h].T @ Z  (0/1 indicator sums j' per b)
DMAs are batched C=8 groups per transfer; host pre-permutes layouts so every
load is a plain 2D [128, C*No] copy.
"""

import os
import sys
import numpy as np

sys.path.insert(0, "/opt/trn_rl_repo")

import ml_dtypes

BF16 = ml_dtypes.bfloat16

# problem constants
D, B, N = 8, 16, 2048
NCORES = 8
NO = N // NCORES  # per-core o-slice width
J = 16            # e's per group
NG = N // J       # e-groups per core
HB = B // 2       # b per half
C = 8             # groups per DMA batch
NB = NG // C      # DMA blocks


def _consts():
    p = np.arange(128)
    m = np.arange(256)
    dmask = (p[:, None] % J == m[None, :] % J).astype(np.float32)
    eh = np.zeros((2, 128, B), dtype=np.float32)
    for h in range(2):
        for bb in range(HB):
            eh[h, bb * J:(bb + 1) * J, h * HB + bb] = 1.0
    return dmask, eh


def host_prep(W, Wlong, Wshort, Xd, delaymap, STDP_frac, signs_pre, use_bf16=True):
    """Host-side prep: signs/A/SF fusion, layout transforms, o-shard."""
    dt = BF16 if use_bf16 else np.float32
    W = np.asarray(W, np.float32)
    frac = np.asarray(STDP_frac, np.float32)
    signs = np.where(W > 0, np.sign(np.asarray(signs_pre, np.float32))[:, None],
                     np.float32(0.0))
    A = (signs * W * (1.0 - frac)).astype(np.float32)
    SF = (signs * frac).astype(np.float32)
    G = (np.asarray(Xd, np.float32) *
         (np.asarray(Wshort, np.float32) + 1.0))  # [D,B,N]

    # gq1[p=(d,j), g, b] = G[d, b, g*J+j]
    Gr = G.reshape(D, B, NG, J)
    gq1f = Gr.transpose(0, 3, 2, 1).reshape(D * J, NG, B)
    gq1 = np.ascontiguousarray(gq1f).astype(dt)


    # dm_r[c][gc, (d,j), s*NO+o] = dm[d, (gc*C+s)*J + j, c*NO+o]
    dmf = np.asarray(delaymap, np.float32)  # [D, N, N]
    wlf = np.asarray(Wlong, np.float32)     # [B, N, N]
    # [D, NB, C, J, N] -> [NB, (D,J), C, N]
    dm5 = dmf.reshape(D, NB, C, J, N).transpose(1, 0, 3, 2, 4)  # [NB, D, J, C, N]
    wl5 = wlf.reshape(2, HB, NB, C, J, N).transpose(2, 0, 1, 4, 3, 5)  # [NB,2,HB,J,C,N]

    # combined A||SF partition-major [128, ET, 2*NO] for full-width DMA
    ET = N // 128

    def pm2(Am, Sm, c):
        sl = slice(c * NO, (c + 1) * NO)
        a = Am[:, sl].reshape(ET, 128, NO).transpose(1, 0, 2)
        s = Sm[:, sl].reshape(ET, 128, NO).transpose(1, 0, 2)
        return np.ascontiguousarray(np.concatenate([a, s], axis=2)).astype(dt)

    ins = []
    for c in range(NCORES):
        sl = slice(c * NO, (c + 1) * NO)
        ins.append({
            "dm": np.ascontiguousarray(
                dm5[:, :, :, :, sl].reshape(NB, 128, C * NO)).astype(dt),
            "wl": np.ascontiguousarray(
                wl5[:, :, :, :, :, sl].reshape(NB, 2, 128, C * NO)).astype(dt),
            "ASF": pm2(A, SF, c),
            "gq1": gq1,
        })
    return ins


def build_nc(use_bf16=True, n_cores=NCORES, no=NO, ng=NG, reps=1):
    """Build the SPMD Bass program (same on all cores)."""
    import concourse.bass as bass
    import concourse.bacc as bacc
    import concourse.mybir as mybir
    import concourse.tile as tile
    from contextlib import ExitStack

    dt_big = mybir.dt.bfloat16 if use_bf16 else mybir.dt.float32
    f32 = mybir.dt.float32
    n = ng * J
    nb = ng // C

    nc = bacc.Bacc("TRN2", target_bir_lowering=False, debug=False,
                   num_devices=n_cores)

    dm = nc.declare_dram_parameter("dm", [nb, 128, C * no], dt_big, isOutput=False).ap()
    wl = nc.declare_dram_parameter("wl", [nb, 2, 128, C * no], dt_big, isOutput=False).ap()
    et = n // 128
    ASF = nc.declare_dram_parameter("ASF", [128, et, 2 * no], dt_big, isOutput=False).ap()
    gq1 = nc.declare_dram_parameter("gq1", [128, ng, B], dt_big, isOutput=False).ap()
    out = nc.declare_dram_parameter("out", [B, no], f32, isOutput=True).ap()

    dmask_np, eh_np = _consts()
    np_dt = BF16 if use_bf16 else np.float32
    eh_dram = nc.inline_tensor(eh_np.astype(np_dt), name="ehc")
    dmask_dram = nc.inline_tensor(dmask_np.astype(np_dt), name="dmaskc")
    # sel[gg][p, (d,j)] = delta_{p, gg*J + (dj % J)} -- replication selectors
    pidx = np.arange(128)
    sel_np = np.stack([
        (pidx[:, None] == (gg * J + (pidx[None, :] % J))) for gg in range(8)
    ]).astype(np_dt)  # [8, 128, 128]
    sel_dram = nc.inline_tensor(sel_np, name="selc")

    def mmdt(ap):
        return ap if use_bf16 else ap.bitcast(mybir.dt.float32r)

    with tile.TileContext(nc) as tc, ExitStack() as ctx:
        res = ctx.enter_context(tc.tile_pool(name="res", bufs=1))
        gq1_sb = res.tile([128, ng, B], dt_big)
        eh_sb = res.tile([128, 2, B], dt_big)
        sel_sb = res.tile([128, 8, 128], dt_big)
        ASF_sb = res.tile([128, et, 2 * no], dt_big)
        dmask_sb = res.tile([128, 256], dt_big)
        nc.sync.dma_start(out=eh_sb[:, :, :], in_=eh_dram.ap().rearrange("h p b -> p h b"))
        nc.sync.dma_start(out=gq1_sb[:, :, :], in_=gq1)
        nc.sync.dma_start(out=sel_sb[:, :, :], in_=sel_dram.ap().rearrange("g p m -> p g m"))
        nc.sync.dma_start(out=dmask_sb[:, :], in_=dmask_dram.ap())
        nc.sync.dma_start(out=ASF_sb[:, :, :], in_=ASF)

        dm_pool = ctx.enter_context(tc.tile_pool(name="dmp", bufs=4))
        gb_pool = ctx.enter_context(tc.tile_pool(name="gbp", bufs=8))
        wl_pool = ctx.enter_context(tc.tile_pool(name="wlp", bufs=4))
        rep_pool = ctx.enter_context(tc.tile_pool(name="repp", bufs=4))
        psum_as = ctx.enter_context(tc.tile_pool(name="psas", bufs=3, space="PSUM"))
        pq_pool = ctx.enter_context(tc.tile_pool(name="pqp", bufs=10))
        hz_pool = ctx.enter_context(tc.tile_pool(name="hzp", bufs=10))
        psum_h = ctx.enter_context(tc.tile_pool(name="psh", bufs=4, space="PSUM"))
        psum_i = ctx.enter_context(tc.tile_pool(name="psi", bufs=1, space="PSUM"))
        out_pool = ctx.enter_context(tc.tile_pool(name="outp", bufs=1))

        I_psum = psum_i.tile([B, no], f32)

        def body(_i=None):
            for gc in range(nb):
                dm_t = dm_pool.tile([128, C * no], dt_big, tag="dm")
                nc.sync.dma_start(out=dm_t[:, :], in_=dm[gc])
                wl_t = [wl_pool.tile([128, C * no], dt_big, tag=f"wl{h}",
                                     name=f"wl_t{h}")
                        for h in range(2)]
                nc.sync.dma_start(out=wl_t[0][:, :], in_=wl[gc, 0])
                nc.sync.dma_start(out=wl_t[1][:, :], in_=wl[gc, 1])

                for s in range(C):
                    g = gc * C + s
                    t, gg = g // 8, g % 8
                    so = slice(s * no, (s + 1) * no)
                    # replicate A/SF 16-row slices across d-blocks on the PE
                    AS_ps = psum_as.tile([128, 2 * no], f32, tag="as")
                    nc.tensor.matmul(AS_ps[:, :], mmdt(sel_sb[:, gg, :]),
                                     mmdt(ASF_sb[:, t, :]),
                                     start=True, stop=True)
                    # block-diag stationary for H': gq1 bcast * dmask (Pool)
                    gb_t = gb_pool.tile([128, 256], dt_big, tag="gb")
                    gq_b = gq1_sb[:, g, :].rearrange(
                        "p (h b) -> p h b", h=2).unsqueeze(3).broadcast_to((128, 2, HB, J))
                    nc.gpsimd.tensor_mul(
                        gb_t.rearrange("p (h b j) -> p h b j", h=2, b=HB),
                        gq_b,
                        dmask_sb.rearrange("p (h b j) -> p h b j", h=2, b=HB))
                    PQ_t = pq_pool.tile([128, 2 * no], dt_big, tag="PQ")
                    dm_b = dm_t[:, so].unsqueeze(1).broadcast_to((128, 2, no))
                    nc.vector.tensor_mul(
                        PQ_t.rearrange("p (r o) -> p r o", r=2), dm_b, AS_ps[:, :])

                    nc.tensor.matmul(I_psum[:, :], mmdt(gq1_sb[:, g, :]),
                                     mmdt(PQ_t[:, :no]),
                                     start=(g == 0), stop=False)

                    Hp = psum_h.tile([128, 2 * no], f32, tag="hp")
                    for h in range(2):
                        nc.tensor.matmul(
                            Hp[:, h * no:(h + 1) * no],
                            mmdt(gb_t[:, h * 128:(h + 1) * 128]),
                            mmdt(PQ_t[:, no:]), start=True, stop=True)
                    Hs = hz_pool.tile([128, 2 * no], dt_big, tag="hs")
                    nc.scalar.copy(Hs[:, :], Hp[:, :])
                    for h in range(2):
                        Z_t = hz_pool.tile([128, no], dt_big, tag="z")
                        nc.gpsimd.tensor_mul(Z_t[:, :], wl_t[h][:, so],
                                             Hs[:, h * no:(h + 1) * no])
                        last = (g == ng - 1) and (h == 1)
                        nc.tensor.matmul(I_psum[:, :], mmdt(eh_sb[:, h, :]),
                                         mmdt(Z_t[:, :]),
                                         start=False, stop=last)

        if reps == 1:
            body()
        else:
            with tc.For_i(0, reps, 1) as _i:
                body(_i)

        I_sb = out_pool.tile([B, no], f32)
        nc.scalar.copy(I_sb[:, :], I_psum[:, :])
        nc.sync.dma_start(out=out, in_=I_sb[:, :])

    nc.compile()
    return nc


_CACHE = {}


def kernel(W, Wlong, Wshort, Xd, delaymap, STDP_frac, signs_pre):
    from concourse.bass_utils import run_bass_kernel_spmd

    use_bf16 = os.environ.get("DS_FP32", "0") != "1"
    ins = host_prep(W, Wlong, Wshort, Xd, delaymap, STDP_frac, signs_pre, use_bf16)
    key = ("nc", use_bf16)
    if key not in _CACHE:
        _CACHE[key] = build_nc(use_bf16)
    nc = _CACHE[key]
    r = run_bass_kernel_spmd(nc, ins, list(range(NCORES)))
    outs = [r.results[c]["out"] for c in range(NCORES)]
    return np.concatenate(outs, axis=1).astype(np.float32)


if __name__ == "__main__":
    pass



# revision 4
# speedup vs baseline: 1.9638x; 1.9638x over previous
"""Trainium2 Bass kernel for nn_DeltaSynapse.

I[b,o] = einsum('beo,dbe,deo,dbe->bo', Weff, Xd, delaymap, Wshort+1)
with Weff[b,e,o] = signs[e,o] * (W[e,o]*(1-frac[e,o]) + Wlong[b,e,o]*frac[e,o])
     signs[e,o] = sign(signs_pre[e]) * (W[e,o] > 0)

Identity: I[b,o] = sum_e H2[b,e,o] * Weff[b,e,o],
          H2[b,e,o] = sum_d G[d,b,e] * dm[d,e,o],  G = Xd*(Wshort+1).

Host computes Weff (= A + SF*Wlong, same bytes as Wlong) and the
block-diagonal stationary gb (core-independent). Device, per o-shard of
no=256 and per e-group g of J=16 e's:
  - H2 matmul per b-half h: Hp[(bb,j'), o] = gb[g,h].T @ dm[:, g-slice]
      (gb[(d,j), (bb,j')] = G[d, h*8+bb, g*16+j]*delta_{j,j'})
  - Z[(bb,j'), (h,o)] = Hp * Weff-tile   (DVE/GpSimd split, PSUM read)
  - Zred: I_ps[16, (s,o)] += eh[h].T @ Z   (512-col matmuls, one PSUM
      accumulation region across all blocks)
Final: DVE tensor_reduce folds the 8 s-chunks -> [16, no] -> DMA out.
DMAs are batched C=8 groups per transfer (dm 0.5MB, wf 1MB, gb 0.5MB).
"""

import os
import sys
import numpy as np

sys.path.insert(0, "/opt/trn_rl_repo")

import ml_dtypes

BF16 = ml_dtypes.bfloat16

# problem constants
D, B, N = 8, 16, 2048
NCORES = 8
NO = N // NCORES  # per-core o-slice width
J = 16            # e's per group
NG = N // J       # e-groups (128)
HB = B // 2       # b per half (8)
C = 8             # groups per DMA block
NB = NG // C      # DMA blocks (16)


def _consts():
    # eh[h, p=(bb,j'), b] = 1 iff b == h*HB+bb  (bb-major partitions)
    eh = np.zeros((2, 128, B), dtype=np.float32)
    for h in range(2):
        for bb in range(HB):
            eh[h, bb * J:(bb + 1) * J, h * HB + bb] = 1.0
    return eh


def host_prep(W, Wlong, Wshort, Xd, delaymap, STDP_frac, signs_pre, use_bf16=True):
    """Host-side prep: Weff fusion, block-diag gb, layout transforms, o-shard."""
    dt = BF16 if use_bf16 else np.float32
    W = np.asarray(W, np.float32)
    frac = np.asarray(STDP_frac, np.float32)
    signs = np.where(W > 0, np.sign(np.asarray(signs_pre, np.float32))[:, None],
                     np.float32(0.0))
    A = signs * W * (1.0 - frac)
    SF = signs * frac
    # Weff[b,e,o] = A[e,o] + SF[e,o]*Wlong[b,e,o]
    Weff = (A[None] + SF[None] * np.asarray(Wlong, np.float32))  # [B,N,N] f32
    G = (np.asarray(Xd, np.float32) *
         (np.asarray(Wshort, np.float32) + 1.0))  # [D,B,N]

    # dm_r[gc, p=(d,j), (s,o)] = dm[d, (gc*C+s)*J+j, c*NO+o]
    dmf = np.asarray(delaymap, np.float32)
    dm5 = dmf.reshape(D, NB, C, J, N).transpose(1, 0, 3, 2, 4)  # [NB,D,J,C,N]

    # wf[gc, p=(bb,j'), (h,s,o)] = Weff[h*HB+bb, (gc*C+s)*J+j', c*NO+o]
    wf6 = Weff.reshape(2, HB, NB, C, J, N).transpose(2, 1, 4, 0, 3, 5)
    # -> [NB, HB, J, 2, C, N];  partition (bb,j') bb-major

    # gb[gc, p=(d,j), (s,h,m=(bb,j'))] = G[d, h*HB+bb, (gc*C+s)*J+j]*delta_{j,j'}
    # core-independent -> build once
    gbar = np.zeros((NB, D, J, C, 2, HB, J), np.float32)
    Gr = G.reshape(D, 2, HB, NB, C, J)  # [d,h,bb,gc,s,j]
    for j in range(J):
        # target [gc, d, s, h, bb]
        gbar[:, :, j, :, :, :, j] = Gr[:, :, :, :, :, j].transpose(3, 0, 4, 1, 2)
    gb = np.ascontiguousarray(gbar.reshape(NB, 128, C * 2 * 128)).astype(dt)

    ins = []
    for c in range(NCORES):
        sl = slice(c * NO, (c + 1) * NO)
        ins.append({
            "dm": np.ascontiguousarray(
                dm5[:, :, :, :, sl].reshape(NB, 128, C * NO)).astype(dt),
            "wf": np.ascontiguousarray(
                wf6[:, :, :, :, :, sl].reshape(NB, 128, 2 * C * NO)).astype(dt),
            "gb": gb,
        })
    return ins


def build_nc(use_bf16=True, n_cores=NCORES, no=NO, ng=NG):
    """Build the SPMD Bass program (same on all cores)."""
    import concourse.bass as bass
    import concourse.bacc as bacc
    import concourse.mybir as mybir
    import concourse.tile as tile
    from contextlib import ExitStack

    dt_big = mybir.dt.bfloat16 if use_bf16 else mybir.dt.float32
    f32 = mybir.dt.float32
    nb = ng // C

    nc = bacc.Bacc("TRN2", target_bir_lowering=False, debug=False,
                   num_devices=n_cores)

    dm = nc.declare_dram_parameter("dm", [nb, 128, C * no], dt_big, isOutput=False).ap()
    wf = nc.declare_dram_parameter("wf", [nb, 128, 2 * C * no], dt_big, isOutput=False).ap()
    gb = nc.declare_dram_parameter("gb", [nb, 128, C * 2 * 128], dt_big, isOutput=False).ap()
    out = nc.declare_dram_parameter("out", [B, no], f32, isOutput=True).ap()

    eh_np = _consts()
    np_dt = BF16 if use_bf16 else np.float32
    eh_dram = nc.inline_tensor(eh_np.astype(np_dt), name="ehc")

    def mmdt(ap):
        return ap if use_bf16 else ap.bitcast(mybir.dt.float32r)

    with tile.TileContext(nc) as tc, ExitStack() as ctx:
        res = ctx.enter_context(tc.tile_pool(name="res", bufs=1))
        eh_sb = res.tile([128, 2, B], dt_big)
        nc.sync.dma_start(out=eh_sb[:, :, :],
                          in_=eh_dram.ap().rearrange("h p b -> p h b"))

        hs_pool = ctx.enter_context(tc.tile_pool(name="hsp", bufs=3))
        dm_pool = ctx.enter_context(tc.tile_pool(name="dmp", bufs=3))
        wf_pool = ctx.enter_context(tc.tile_pool(name="wfp", bufs=3))
        gb_pool = ctx.enter_context(tc.tile_pool(name="gbp", bufs=3))
        z_pool = ctx.enter_context(tc.tile_pool(name="zp", bufs=3))
        psum_h = ctx.enter_context(tc.tile_pool(name="psh", bufs=4, space="PSUM"))
        psum_i = ctx.enter_context(tc.tile_pool(name="psi", bufs=1, space="PSUM"))
        out_pool = ctx.enter_context(tc.tile_pool(name="outp", bufs=1))

        # persistent accumulator [16, (s,o)] = 4 PSUM banks
        I_ps = psum_i.tile([B, C * no], f32)

        # Z-mul engine split: GpSimd is ~1.9x slower per element than DVE.
        # 84 DVE : 44 GpSimd over 128 groups -> ~5.25:2.75 per 8 -> use
        # s in {2,5,7} on GpSimd (3/8 = 48 groups) ~ close enough; tune later.
        GS_SLOTS = (2, 5, 7)

        for gc in range(nb):
            dm_t = dm_pool.tile([128, C * no], dt_big, tag="dm")
            nc.sync.dma_start(out=dm_t[:, :], in_=dm[gc])
            wf_t = wf_pool.tile([128, 2 * C * no], dt_big, tag="wf")
            nc.sync.dma_start(out=wf_t[:, :], in_=wf[gc])
            gb_t = gb_pool.tile([128, C * 2 * 128], dt_big, tag="gb")
            nc.sync.dma_start(out=gb_t[:, :], in_=gb[gc])

            gb_v = gb_t.rearrange("p (s h m) -> p s h m", s=C, h=2)
            wf_v = wf_t.rearrange("p (h s o) -> p h s o", h=2, s=C)
            Z_t = z_pool.tile([128, 2 * C * no], dt_big, tag="z")
            Z_v = Z_t.rearrange("p (h s o) -> p h s o", h=2, s=C)

            for s in range(C):
                Hp = psum_h.tile([128, 2 * no], f32, tag="hp")
                so = slice(s * no, (s + 1) * no)
                for h in range(2):
                    nc.tensor.matmul(Hp[:, h * no:(h + 1) * no],
                                     mmdt(gb_v[:, s, h, :]),
                                     mmdt(dm_t[:, so]),
                                     start=True, stop=True)
                if s in GS_SLOTS:
                    # GpSimd cannot read PSUM: ACT evacuates to SBUF bf16
                    Hs = hs_pool.tile([128, 2 * no], dt_big, tag="hs")
                    nc.scalar.copy(Hs[:, :], Hp[:, :])
                    nc.gpsimd.tensor_mul(Z_v[:, :, s, :],
                                         wf_v[:, :, s, :],
                                         Hs.rearrange("p (h o) -> p h o", h=2))
                else:
                    nc.vector.tensor_mul(Z_v[:, :, s, :],
                                         wf_v[:, :, s, :],
                                         Hp.rearrange("p (h o) -> p h o", h=2))

            Z_h = Z_t.rearrange("p (h x) -> p h x", h=2)
            for h in range(2):
                for k in range(4):  # 512-col chunks, one PSUM bank each
                    ks = slice(k * 512, (k + 1) * 512)
                    nc.tensor.matmul(I_ps[:, ks],
                                     mmdt(eh_sb[:, h, :]),
                                     mmdt(Z_h[:, h, ks]),
                                     start=(gc == 0 and h == 0),
                                     stop=(gc == nb - 1 and h == 1))

        # fold s-chunks: [16, (s,o)] viewed as [16, o, s] -> reduce X -> [16, o]
        I_sb = out_pool.tile([B, no], f32)
        nc.vector.tensor_reduce(I_sb[:, :],
                                I_ps.rearrange("b (s o) -> b o s", s=C),
                                axis=mybir.AxisListType.X,
                                op=mybir.AluOpType.add)
        nc.sync.dma_start(out=out, in_=I_sb[:, :])

    nc.compile()
    return nc


_CACHE = {}


def kernel(W, Wlong, Wshort, Xd, delaymap, STDP_frac, signs_pre):
    from concourse.bass_utils import run_bass_kernel_spmd

    use_bf16 = os.environ.get("DS_FP32", "0") != "1"
    ins = host_prep(W, Wlong, Wshort, Xd, delaymap, STDP_frac, signs_pre, use_bf16)
    key = ("nc", use_bf16)
    if key not in _CACHE:
        _CACHE[key] = build_nc(use_bf16)
    nc = _CACHE[key]
    r = run_bass_kernel_spmd(nc, ins, list(range(NCORES)))
    outs = [r.results[c]["out"] for c in range(NCORES)]
    return np.concatenate(outs, axis=1).astype(np.float32)


if __name__ == "__main__":
    pass
